# revision 4
# baseline (speedup 1.0000x reference)
"""KNN-attention transformer block on 8 TRN2 NeuronCores (Bass/Tile).

Sharding: pure data-parallel over tokens (B*S = 4096 -> 512 tokens/core),
memory bank + weights replicated per core (no collectives).  db_keys is fed
twice: natural layout for the gather and host-transposed [DM, M] so the
scores matmul streams it as the fp32r moving operand at full PE rate.

Per core:
  LN1 -> qT (fp32r matmuls) -> scores vs all 32768 keys, consumed straight
  out of PSUM by a per-512-chunk DVE max8/max_index scan -> merge top-16 via
  match_replace + float-encoded global indices -> per-token indirect-DMA
  gather of the 16 db rows -> per-head softmax attention on DVE in
  token-major layout -> c_proj -> LN2 -> MLP (gelu fused into PSUM evict).
"""
import sys

sys.path.insert(0, "/opt/trn_rl_repo")

import numpy as np
import concourse.bass as bass
import concourse.tile as tile
from concourse import bacc, mybir
from concourse.bass_utils import run_bass_kernel_spmd
from concourse.masks import make_identity

f32 = mybir.dt.float32
f32r = mybir.dt.float32r
u16 = mybir.dt.uint16
u32 = mybir.dt.uint32
AF = mybir.ActivationFunctionType
ALU = mybir.AluOpType
AX = mybir.AxisListType

B, S, DM, H, HD, K, M = 2, 2048, 768, 12, 64, 16, 32768
NCORES = 8
TOK = (B * S) // NCORES          # 512 tokens per core
TB = TOK // 128                  # 4 token blocks
DC = DM // 128                   # 6 dmodel chunks
MB = 512                         # scores chunk width
NMB = M // MB                    # 64 chunks
NCAND = NMB * 8                  # 512 candidates/token
FF = 4 * DM
FC = FF // 128                   # 24 ff chunks
NEG = -3.0e38


def _ts(i, n):
    return slice(i * n, (i + 1) * n)


def build_program():
    nc = bacc.Bacc("TRN2", target_bir_lowering=False, debug=False)

    xs_d = nc.dram_tensor("xs", (TOK, DM), f32, kind="ExternalInput").ap()
    dbt_d = nc.dram_tensor("dbt", (DM, M), f32r, kind="ExternalInput").ap()
    dbk_d = nc.dram_tensor("dbk", (M, DM), f32, kind="ExternalInput").ap()
    dbv_d = nc.dram_tensor("dbv", (M, DM), f32, kind="ExternalInput").ap()
    ln1g_d = nc.dram_tensor("ln1g", (DM,), f32, kind="ExternalInput").ap()
    ln1b_d = nc.dram_tensor("ln1b", (DM,), f32, kind="ExternalInput").ap()
    caw_d = nc.dram_tensor("caw", (DM, DM), f32r, kind="ExternalInput").ap()
    cab_d = nc.dram_tensor("cab", (DM,), f32, kind="ExternalInput").ap()
    cpw_d = nc.dram_tensor("cpw", (DM, DM), f32r, kind="ExternalInput").ap()
    cpb_d = nc.dram_tensor("cpb", (DM,), f32, kind="ExternalInput").ap()
    ln2g_d = nc.dram_tensor("ln2g", (DM,), f32, kind="ExternalInput").ap()
    ln2b_d = nc.dram_tensor("ln2b", (DM,), f32, kind="ExternalInput").ap()
    fcw_d = nc.dram_tensor("fcw", (DM, FF), f32r, kind="ExternalInput").ap()
    fcb_d = nc.dram_tensor("fcb", (FF,), f32, kind="ExternalInput").ap()
    pjw_d = nc.dram_tensor("pjw", (FF, DM), f32r, kind="ExternalInput").ap()
    pjb_d = nc.dram_tensor("pjb", (DM,), f32, kind="ExternalInput").ap()
    out_d = nc.dram_tensor("out", (TOK, DM), f32, kind="ExternalOutput").ap()

    with tile.TileContext(nc) as tc:
        with (
            tc.tile_pool(name="const", bufs=1) as cpool,
            tc.tile_pool(name="wts", bufs=1) as wpool,
            tc.tile_pool(name="act", bufs=1) as apool,
            tc.tile_pool(name="big", bufs=1) as bigpool,
            tc.tile_pool(name="dbt", bufs=2) as dbpool,
            tc.tile_pool(name="scr", bufs=2) as spool,
            tc.tile_pool(name="scr1", bufs=1) as s1pool,
            tc.tile_pool(name="gat", bufs=3) as gpool,
            tc.tile_pool(name="psmm", bufs=3, space="PSUM") as psmm,
            tc.tile_pool(name="pstp", bufs=1, space="PSUM") as pstp,
            tc.tile_pool(name="psbig", bufs=2, space="PSUM") as psbig,
        ):
            # ---------------- constants ----------------
            ident = cpool.tile([128, 128], f32, tag="ident")
            make_identity(nc, ident[:])

            def bcast_load(name, dram_ap, width=DM):
                t = cpool.tile([128, width], f32, tag=name, name=name)
                src = bass.AP(dram_ap.tensor, 0, [[0, 128], [1, width]])
                nc.sync.dma_start(out=t[:], in_=src)
                return t

            ln1g_t = bcast_load("ln1g", ln1g_d)
            ln1b_t = bcast_load("ln1b", ln1b_d)
            ln2g_t = bcast_load("ln2g", ln2g_d)
            ln2b_t = bcast_load("ln2b", ln2b_d)
            cpb_t = bcast_load("cpb", cpb_d)
            pjb_t = bcast_load("pjb", pjb_d)

            cab_t = cpool.tile([128, DC], f32, tag="cab")
            nc.sync.dma_start(
                out=cab_t[:], in_=bass.AP(cab_d.tensor, 0, [[1, 128], [128, DC]])
            )
            fcb_t = cpool.tile([128, FC], f32, tag="fcb")
            nc.sync.dma_start(
                out=fcb_t[:], in_=bass.AP(fcb_d.tensor, 0, [[1, 128], [128, FC]])
            )

            bases_u = s1pool.tile([128, NCAND], u32, tag="mg_mask")
            nc.gpsimd.iota(
                bases_u[:], pattern=[[MB, NMB], [0, 8]], base=1,
                channel_multiplier=0,
            )
            bases_f = cpool.tile([128, NCAND], f32, tag="bases_f")
            nc.vector.tensor_copy(bases_f[:], bases_u[:])

            cpw_t = wpool.tile([128, DC, DM], f32r, tag="cpw")
            nc.sync.dma_start(
                out=cpw_t[:], in_=cpw_d.rearrange("(c p) n -> p c n", p=128)
            )

            # ---------------- helpers ----------------
            def layernorm(x_ap, g_t, b_t, out_ap, sq_scratch):
                # out_ap may alias sq_scratch's storage only via the Square
                # scratch write (consumed solely through accum_out).
                mu = s1pool.tile([128, 1], f32, tag="ln_mu")
                nc.vector.tensor_reduce(mu[:], x_ap, axis=AX.X, op=ALU.add)
                nc.vector.tensor_scalar(
                    out=mu[:], in0=mu[:], scalar1=1.0 / DM, scalar2=None,
                    op0=ALU.mult,
                )
                xc = s1pool.tile([128, DM], f32, tag="ln_xc")
                nc.vector.tensor_scalar(
                    out=xc[:], in0=x_ap, scalar1=mu[:], scalar2=None,
                    op0=ALU.subtract,
                )
                ssum = s1pool.tile([128, 1], f32, tag="ln_ss")
                nc.scalar.activation(
                    sq_scratch, xc[:], AF.Square, accum_out=ssum[:]
                )
                rstd = s1pool.tile([128, 1], f32, tag="ln_rstd")
                nc.vector.tensor_scalar(
                    out=rstd[:], in0=ssum[:], scalar1=1.0 / DM, scalar2=1e-5,
                    op0=ALU.mult, op1=ALU.add,
                )
                nc.scalar.activation(rstd[:], rstd[:], AF.Sqrt)
                nc.vector.reciprocal(rstd[:], rstd[:])
                nc.vector.tensor_scalar(
                    out=xc[:], in0=xc[:], scalar1=rstd[:], scalar2=None,
                    op0=ALU.mult,
                )
                nc.vector.tensor_tensor(
                    out=xc[:], in0=xc[:], in1=g_t[:], op=ALU.mult
                )
                nc.vector.tensor_tensor(
                    out=out_ap, in0=xc[:], in1=b_t[:], op=ALU.add
                )

            def transpose128(src_ap, dst_ap):
                tp = pstp.tile([128, 128], f32, tag="tp")
                nc.tensor.transpose(tp[:], src_ap, ident[:])
                nc.vector.tensor_copy(dst_ap, tp[:])

            # ---------------- phase 1: LN1 -> h1T -> qT ----------------
            hT = apool.tile([128, DC, TOK], f32r, tag="hT", name="h1T")
            for tb in range(TB):
                x_t = spool.tile([128, DM], f32, tag="xio", name="x_in")
                nc.sync.dma_start(out=x_t[:], in_=xs_d[_ts(tb, 128), :])
                h1 = s1pool.tile([128, DM], f32, tag="h1")
                layernorm(x_t[:], ln1g_t, ln1b_t, h1[:], h1[:])
                for c in range(DC):
                    transpose128(
                        h1[:, _ts(c, 128)], hT[:, c, tb * 128:(tb + 1) * 128]
                    )

            qT = apool.tile([128, DC, TOK], f32r, tag="qT")
            for cq in range(DC):
                caw_t = spool.tile([128, DC, 128], f32r, tag="wstream",
                                   name="caw_t")
                nc.sync.dma_start(
                    out=caw_t[:],
                    in_=caw_d.rearrange("(c p) n -> p c n", p=128)[
                        :, :, _ts(cq, 128)
                    ],
                )
                qps = psmm.tile([128, TOK], f32, tag="mm512", name="qps")
                for c in range(DC):
                    nc.tensor.matmul(
                        qps[:], caw_t[:, c, :], hT[:, c, :],
                        start=(c == 0), stop=(c == DC - 1),
                    )
                nc.vector.tensor_scalar(
                    out=qT[:, cq, :], in0=qps[:],
                    scalar1=cab_t[:, cq:cq + 1], scalar2=None, op0=ALU.add,
                )

            # ---------------- phase 2: scores + per-chunk top-8 ----------
            cand_v = apool.tile([128, TB * NCAND], f32, tag="cand_v")
            cand_i = apool.tile([128, TB * NCAND], u16, tag="cand_i")
            for mb in range(NMB):
                dbt_t = dbpool.tile([128, DC, MB], f32r, tag="dbt")
                nc.sync.dma_start(
                    out=dbt_t[:],
                    in_=dbt_d.rearrange("(c p) m -> p c m", p=128)[
                        :, :, _ts(mb, MB)
                    ],
                )
                for tb in range(TB):
                    scps = psmm.tile([128, MB], f32, tag="mm512", name="scps")
                    for c in range(DC):
                        nc.tensor.matmul(
                            scps[:],
                            qT[:, c, tb * 128:(tb + 1) * 128],
                            dbt_t[:, c, :],
                            start=(c == 0), stop=(c == DC - 1),
                        )
                    base = tb * NCAND + mb * 8
                    nc.vector.max(cand_v[:, base:base + 8], scps[:])
                    nc.vector.max_index(
                        cand_i[:, base:base + 8], cand_v[:, base:base + 8],
                        scps[:],
                    )

            # ---------------- phase 3: merge -> top-16 global indices -----
            idx_u = apool.tile([128, TB * K], u32, tag="idx_u")
            for tb in range(TB):
                cv = cand_v[:, _ts(tb, NCAND)]
                posg = s1pool.tile([128, NCAND], f32, tag="mg_posg")
                nc.vector.tensor_copy(posg[:], cand_i[:, _ts(tb, NCAND)])
                nc.vector.tensor_tensor(
                    out=posg[:], in0=posg[:], in1=bases_f[:], op=ALU.add
                )
                m8 = s1pool.tile([128, 16], f32, tag="mg_m8")
                nc.vector.max(m8[:, 0:8], cv)
                nc.vector.match_replace(cv, m8[:, 0:8], cv, NEG)
                nc.vector.max(m8[:, 8:16], cv)
                nc.vector.match_replace(cv, m8[:, 8:16], cv, NEG)
                mask = s1pool.tile([128, NCAND], f32, tag="mg_mask",
                                   name="mask")
                nc.vector.tensor_scalar(
                    out=mask[:], in0=cv, scalar1=NEG, scalar2=None,
                    op0=ALU.is_equal,
                )
                nc.vector.tensor_tensor(
                    out=mask[:], in0=mask[:], in1=posg[:], op=ALU.mult
                )
                nc.vector.tensor_scalar(
                    out=mask[:], in0=mask[:], scalar1=1.0, scalar2=None,
                    op0=ALU.subtract,
                )
                i16 = s1pool.tile([128, 16], f32, tag="mg_i16")
                nc.vector.max(i16[:, 0:8], mask[:])
                nc.vector.match_replace(mask[:], i16[:, 0:8], mask[:], -1.0)
                nc.vector.max(i16[:, 8:16], mask[:])
                nc.vector.tensor_copy(idx_u[:, _ts(tb, K)], i16[:])

            # ------- phase 4+5: gather, attention, c_proj, LN2, h2T -------
            # hT tag is re-used: h1T is dead after the qkv matmuls.
            h2T = apool.tile([128, DC, TOK], f32r, tag="hT", name="h2T")
            res2 = apool.tile([128, TB, DM], f32, tag="res2")
            for tb in range(TB):
                q_sb = s1pool.tile([128, DM], f32, tag="q_sb")
                for c in range(DC):
                    transpose128(
                        qT[:, c, tb * 128:(tb + 1) * 128].bitcast(f32),
                        q_sb[:, _ts(c, 128)],
                    )
                logit = s1pool.tile([128, H, K], f32, tag="at_logit")
                for m in range(K):
                    mk = gpool.tile([128, DM], f32, tag="gatrow", name="mk")
                    nc.gpsimd.indirect_dma_start(
                        out=mk[:], out_offset=None, in_=dbk_d,
                        in_offset=bass.IndirectOffsetOnAxis(
                            ap=idx_u[:, tb * K + m:tb * K + m + 1], axis=0
                        ),
                    )
                    prod = spool.tile([128, DM], f32, tag="at_prod")
                    nc.vector.tensor_tensor(
                        out=prod[:], in0=mk[:], in1=q_sb[:], op=ALU.mult
                    )
                    nc.vector.tensor_reduce(
                        logit[:, :, m],
                        prod[:].rearrange("p (h d) -> p h d", h=H),
                        axis=AX.X, op=ALU.add,
                    )
                ex = s1pool.tile([128, H, K], f32, tag="at_ex")
                nc.scalar.activation(ex[:], logit[:], AF.Exp, scale=0.125)
                ssum = s1pool.tile([128, H], f32, tag="at_ssum")
                nc.vector.tensor_reduce(ssum[:], ex[:], axis=AX.X, op=ALU.add)
                nc.vector.reciprocal(ssum[:], ssum[:])
                aw = s1pool.tile([128, H, K], f32, tag="at_aw")
                nc.vector.tensor_tensor(
                    out=aw[:], in0=ex[:],
                    in1=ssum[:].broadcast_to((128, H, K)), op=ALU.mult,
                )
                # weighted value sum, two groups of 8 to halve p2 footprint
                attn = s1pool.tile([128, DM], f32, tag="h1", name="attn")
                for g in range(2):
                    p2 = bigpool.tile([128, DM, 8], f32, tag="big",
                                      name=f"p2_{tb}_{g}")
                    for m8 in range(8):
                        m = g * 8 + m8
                        mv = gpool.tile([128, DM], f32, tag="gatrow",
                                        name="mv")
                        nc.gpsimd.indirect_dma_start(
                            out=mv[:], out_offset=None, in_=dbv_d,
                            in_offset=bass.IndirectOffsetOnAxis(
                                ap=idx_u[:, tb * K + m:tb * K + m + 1],
                                axis=0,
                            ),
                        )
                        nc.vector.tensor_tensor(
                            out=p2[:, :, m8].rearrange(
                                "p (h d) -> p h d", h=H
                            ),
                            in0=mv[:].rearrange("p (h d) -> p h d", h=H),
                            in1=aw[:, :, m:m + 1].broadcast_to((128, H, HD)),
                            op=ALU.mult,
                        )
                    if g == 0:
                        nc.vector.tensor_reduce(
                            attn[:], p2[:], axis=AX.X, op=ALU.add
                        )
                    else:
                        part = spool.tile([128, DM], f32, tag="at_prod",
                                          name="part1")
                        nc.vector.tensor_reduce(
                            part[:], p2[:], axis=AX.X, op=ALU.add
                        )
                        nc.vector.tensor_tensor(
                            out=attn[:], in0=attn[:], in1=part[:], op=ALU.add
                        )

                attnT = s1pool.tile([128, DC, 128], f32r, tag="attnT")
                for c in range(DC):
                    transpose128(attn[:, _ts(c, 128)], attnT[:, c, :])
                cp = psbig.tile([128, DM], f32, tag="big768", name="cp")
                for c in range(DC):
                    nc.tensor.matmul(
                        cp[:, 0:512], attnT[:, c, :], cpw_t[:, c, 0:512],
                        start=(c == 0), stop=(c == DC - 1),
                    )
                for c in range(DC):
                    nc.tensor.matmul(
                        cp[:, 512:768], attnT[:, c, :], cpw_t[:, c, 512:768],
                        start=(c == 0), stop=(c == DC - 1),
                    )
                x_t = spool.tile([128, DM], f32, tag="xio", name="x_in2")
                nc.sync.dma_start(out=x_t[:], in_=xs_d[_ts(tb, 128), :])
                nc.vector.tensor_tensor(
                    out=x_t[:], in0=x_t[:], in1=cpb_t[:], op=ALU.add
                )
                nc.vector.tensor_tensor(
                    out=res2[:, tb, :], in0=cp[:], in1=x_t[:], op=ALU.add
                )
                h2 = s1pool.tile([128, DM], f32, tag="h1", name="h2")
                layernorm(res2[:, tb, :], ln2g_t, ln2b_t, h2[:], h2[:])
                for c in range(DC):
                    transpose128(
                        h2[:, _ts(c, 128)], h2T[:, c, tb * 128:(tb + 1) * 128]
                    )

            # ---------------- phase 6: MLP in two token halves ------------
            for half in range(2):
                hs = slice(half * 256, (half + 1) * 256)
                aT = bigpool.tile([128, FC, 256], f32r, tag="big",
                                  name=f"aT_{half}")
                for nf in range(FC):
                    fcw_t = spool.tile([128, DC, 128], f32r, tag="wstream",
                                       name="fcw_t")
                    nc.sync.dma_start(
                        out=fcw_t[:],
                        in_=fcw_d.rearrange("(c p) n -> p c n", p=128)[
                            :, :, _ts(nf, 128)
                        ],
                    )
                    fps = psmm.tile([128, 256], f32, tag="mm512", name="fps")
                    for c in range(DC):
                        nc.tensor.matmul(
                            fps[:], fcw_t[:, c, :], h2T[:, c, hs],
                            start=(c == 0), stop=(c == DC - 1),
                        )
                    nc.scalar.activation(
                        aT[:, nf, :], fps[:], AF.Gelu_apprx_tanh,
                        bias=fcb_t[:, nf:nf + 1], scale=1.0,
                    )

                pj_ps = [
                    psbig.tile(
                        [128, DM], f32, tag="big768", name=f"pj_{half}_{j}"
                    )
                    for j in range(2)
                ]
                for kf in range(FC):
                    pjw_t = spool.tile([128, DM], f32r, tag="pjw_t")
                    nc.sync.dma_start(out=pjw_t[:], in_=pjw_d[_ts(kf, 128), :])
                    for j in range(2):
                        nc.tensor.matmul(
                            pj_ps[j][:, 0:512],
                            aT[:, kf, j * 128:(j + 1) * 128],
                            pjw_t[:, 0:512],
                            start=(kf == 0), stop=(kf == FC - 1),
                        )
                        nc.tensor.matmul(
                            pj_ps[j][:, 512:768],
                            aT[:, kf, j * 128:(j + 1) * 128],
                            pjw_t[:, 512:768],
                            start=(kf == 0), stop=(kf == FC - 1),
                        )
                for j in range(2):
                    tb = half * 2 + j
                    ob = spool.tile([128, DM], f32, tag="xio", name="ob")
                    nc.vector.tensor_tensor(
                        out=ob[:], in0=res2[:, tb, :], in1=pjb_t[:],
                        op=ALU.add,
                    )
                    nc.vector.tensor_tensor(
                        out=ob[:], in0=ob[:], in1=pj_ps[j][:], op=ALU.add
                    )
                    nc.sync.dma_start(out=out_d[_ts(tb, 128), :], in_=ob[:])

    nc.compile()
    return nc


_PROG = None


def kernel(**inputs):
    global _PROG
    if _PROG is None:
        _PROG = build_program()
    nc = _PROG

    x = np.ascontiguousarray(
        np.asarray(inputs["previous_hidden"], dtype=np.float32).reshape(
            B * S, DM
        )
    )
    dbk = np.ascontiguousarray(np.asarray(inputs["db_keys"], np.float32))
    dbv = np.ascontiguousarray(np.asarray(inputs["db_values"], np.float32))
    dbt = np.ascontiguousarray(dbk.T)
    caw_q = np.ascontiguousarray(np.asarray(inputs["c_attn_w"], np.float32)[:, :DM])
    cab_q = np.ascontiguousarray(np.asarray(inputs["c_attn_b"], np.float32)[:DM])

    shared = dict(
        dbt=dbt, dbk=dbk, dbv=dbv,
        ln1g=np.asarray(inputs["ln1_g"], np.float32),
        ln1b=np.asarray(inputs["ln1_b"], np.float32),
        caw=caw_q, cab=cab_q,
        cpw=np.asarray(inputs["c_proj_w"], np.float32),
        cpb=np.asarray(inputs["c_proj_b"], np.float32),
        ln2g=np.asarray(inputs["ln2_g"], np.float32),
        ln2b=np.asarray(inputs["ln2_b"], np.float32),
        fcw=np.asarray(inputs["fc_w"], np.float32),
        fcb=np.asarray(inputs["fc_b"], np.float32),
        pjw=np.asarray(inputs["proj_w"], np.float32),
        pjb=np.asarray(inputs["proj_b"], np.float32),
    )
    in_maps = [
        dict(shared, xs=np.ascontiguousarray(x[_ts(i, TOK), :]))
        for i in range(NCORES)
    ]
    res = run_bass_kernel_spmd(nc, in_maps, core_ids=list(range(NCORES)))
    out = np.concatenate([r["out"] for r in res.results], axis=0)
    return out.reshape(B, S, DM)
